# revision 33
# baseline (speedup 1.0000x reference)
"""Multi-head self-attention (B=2, S=2048, D=1024, H=16) on 8 Trainium2 NeuronCores.

Sharding: batch x head-group. Core c = b*4 + g handles batch b and heads 4g..4g+3
(Megatron-style TP: Wq/Wk/Wv column-sharded, Wo row-sharded; partial outputs
summed on the host).

Per-core kernel layout ("T-layout": sequence on the free dim everywhere),
all matmul operands bf16, PSUM accumulation fp32:
  inputs (host-prepared):  xt [1024, 2048] = x[b].T;  wq/wk/wv [1024, 256]
  (scale-folded, transposed);  wo [256, 1024] (scale-folded, transposed)
  QT/KT = (w.T @ xt) [256, 2048]        d' on partitions, heads pair-stacked
  V     = (xt.T @ wv) [2048, 260]       natural layout + ones column per head
  scoresT[k, q] = KT_h-slices.T @ QT_h  per head, k on partitions (row-tiled
                                        T0/T8 pair: both heads of a pair run
                                        concurrently on the PE)
  expT = exp(scoresT / 8)               (no max subtraction: |scores| <~ 2)
  ctxT_aug[d+1, q] = [V_h | 1].T @ expT K=128 accumulation in one PSUM bank;
                                        row 64 = softmax denominator
  ctxT = ctxT_aug[0:64] * (1/denom)     recip on DVE, denom row broadcast via
                                        gpsimd partition_broadcast
  outT_partial = wo.T @ ctxT [1024, 2048]
Host: out[b] = sum_g outT[b, g].T

Pipeline structure: phase-2 score PSUM double-buffered so the Exp ACTIVATEs
(the critical path, ~128 x [128,1024]) stream back-to-back on the Scalar
engine while the PE interleaves scores/ctx with "filler" work (V projection,
remaining QT tiles, per-n output projection) to stay HAM-warm.
"""
import sys

sys.path.insert(0, "/opt/trn_rl_repo")

import numpy as np
import ml_dtypes

import concourse.bass as bass
import concourse.tile as tile
from concourse import bacc, mybir
from concourse.bass_utils import run_bass_kernel_spmd

F32 = mybir.dt.float32
BF16 = mybir.dt.bfloat16
NP_BF16 = ml_dtypes.bfloat16

S = 2048          # sequence length per batch
D = 1024          # embedding dim
HG = 4            # heads per core
HD = 64           # head dim
GC = HG * HD      # group cols = 256
P = 128
NQ = 4            # q chunks of 512
QW = 512          # q chunk width
NKC = 16          # key-position chunks of 128
KO = 8            # contraction chunks of 128 over D
VW = HD + 1       # V columns per head incl. ones column

_NC_CACHE = {}
DEBUG_DUMPS = False


def _build():
    if "nc" in _NC_CACHE:
        return _NC_CACHE["nc"]
    nc = bacc.Bacc(trn_type="TRN2", target_bir_lowering=False, debug=False)
    xt_d = nc.dram_tensor("xt", [D, S], BF16, kind="ExternalInput")
    wq_d = nc.dram_tensor("wq", [D, GC], BF16, kind="ExternalInput")
    wk_d = nc.dram_tensor("wk", [D, GC], BF16, kind="ExternalInput")
    wv_d = nc.dram_tensor("wv", [D, GC], BF16, kind="ExternalInput")
    wo_d = nc.dram_tensor("wo", [GC, D], BF16, kind="ExternalInput")
    out_d = nc.dram_tensor("out_t", [D, S], F32, kind="ExternalOutput")
    dbg = None
    if DEBUG_DUMPS:
        dbg = {
            "dbg_qt": nc.dram_tensor("dbg_qt", [P, 2, S], BF16,
                                     kind="ExternalOutput"),
            "dbg_kt": nc.dram_tensor("dbg_kt", [P, 2, S], BF16,
                                     kind="ExternalOutput"),
            "dbg_va": nc.dram_tensor("dbg_va", [P, NKC, HG * VW], BF16,
                                     kind="ExternalOutput"),
            "dbg_ct": nc.dram_tensor("dbg_ct", [P, 2, S], BF16,
                                     kind="ExternalOutput"),
            "dbg_dn": nc.dram_tensor("dbg_dn", [P, 4, QW], F32,
                                     kind="ExternalOutput"),
            "dbg_bc": nc.dram_tensor("dbg_bc", [P, 2, QW], F32,
                                     kind="ExternalOutput"),
            "dbg_ex": nc.dram_tensor("dbg_ex", [P, 2, QW], BF16,
                                     kind="ExternalOutput"),
        }
    with tile.TileContext(nc) as tc:
        _emit(nc, tc, xt_d, wq_d, wk_d, wv_d, wo_d, out_d, dbg)
    nc.compile()
    _NC_CACHE["nc"] = nc
    return nc


def _emit(nc, tc, xt_d, wq_d, wk_d, wv_d, wo_d, out_d, dbg=None):
    mult = mybir.AluOpType.mult
    with tc.tile_pool(name="big", bufs=1) as big, \
         tc.tile_pool(name="ex", bufs=10) as ex_pool, \
         tc.tile_pool(name="dn", bufs=2) as dn_pool, \
         tc.tile_pool(name="bcn", bufs=2) as bc_pool, \
         tc.tile_pool(name="ot", bufs=2) as ot_pool, \
         tc.tile_pool(name="ps_s", bufs=2, space="PSUM") as ps_s, \
         tc.tile_pool(name="ps_g", bufs=2, space="PSUM") as ps_g, \
         tc.tile_pool(name="ps_c", bufs=1, space="PSUM") as ps_c:

        # ---- persistent SBUF tensors ----
        xs = big.tile([P, KO, S], BF16)          # x.T  [d_in(128) x ko x s]
        wqs = big.tile([P, KO, GC], BF16)
        wks = big.tile([P, KO, GC], BF16)
        wvs = big.tile([P, KO, GC], BF16)
        wo_sb = big.tile([P, 2, D], BF16)        # [d'(128) x chunk x e]
        qt = big.tile([P, 2, S], BF16)           # head h at parts (h%2)*64, chunk h//2
        kt = big.tile([P, 2, S], BF16)
        va = big.tile([P, NKC, HG * VW], BF16)   # V natural + ones col per head
        ct = big.tile([P, 2, S], BF16)           # normalized ctxT, same layout as qt

        # ---- input DMAs (all upfront; transfers share DMA bandwidth and
        # finish ~17us in, which matches the PE's phase-1 warm-up anyway) ----
        xt_r = xt_d.rearrange("(ko p) s -> p ko s", p=P)
        nc.sync.dma_start(wqs[:], wq_d.rearrange("(ko p) m -> p ko m", p=P))
        nc.sync.dma_start(xs[:, :, 0:QW], xt_r[:, :, 0:QW])
        nc.sync.dma_start(wks[:], wk_d.rearrange("(ko p) m -> p ko m", p=P))
        nc.sync.dma_start(wvs[:], wv_d.rearrange("(ko p) m -> p ko m", p=P))
        for nn in range(1, NQ):
            nc.sync.dma_start(xs[:, :, nn * QW:(nn + 1) * QW],
                              xt_r[:, :, nn * QW:(nn + 1) * QW])
        nc.sync.dma_start(wo_sb[:], wo_d.rearrange("(c p) e -> p c e", p=P))

        # ones columns of V_aug (col HD of each VW-wide head block): bf16 1.0
        va_h = va[:].rearrange("p s (h c) -> p s h c", c=VW)
        for h in range(HG):
            nc.vector.memset(
                va_h[:, :, h, HD:HD + 1].bitcast(mybir.dt.uint16), 0x3F80)

        # ---- trickle scheduler: projection / V / phase-3 work chopped into
        # ~2-matmul units with (not_before, deadline) stream slots; units are
        # force-flushed before the scores that depend on them and otherwise
        # paced a couple per slot so the PE never bunches filler work in
        # front of the Scalar engine's exp stream ----
        ot_ref = [None]
        out_r = out_d.rearrange("(m p) q -> p m q", p=P)

        def proj_units(w_sb, dst, m, n, nb, dl):
            """QT/KT tile [128 x 512] as 4 units of 2 ko-chunks each."""
            h = [None]

            def unit(j):
                def run():
                    if j == 0:
                        h[0] = ps_g.tile([P, QW], F32, tag="g", name="g")
                    for ko in (2 * j, 2 * j + 1):
                        nc.tensor.matmul(
                            h[0][:], w_sb[:, ko, m * P:(m + 1) * P],
                            xs[:, ko, n * QW:(n + 1) * QW],
                            start=(ko == 0), stop=(ko == KO - 1))
                    if j == 3:
                        nc.vector.tensor_copy(
                            dst[:, m, n * QW:(n + 1) * QW], h[0][:])
                return run
            return [(nb, dl, unit(j)) for j in range(4)]

        def v_units(sc):
            """V natural tile [128 x 256] as 2 units of 4 ko-chunks."""
            h = [None]

            def unit(j):
                def run():
                    if j == 0:
                        h[0] = ps_g.tile([P, QW], F32, tag="g", name="g")
                    for ko in range(4 * j, 4 * j + 4):
                        nc.tensor.matmul(
                            h[0][:, :GC], xs[:, ko, sc * P:(sc + 1) * P],
                            wvs[:, ko, :],
                            start=(ko == 0), stop=(ko == KO - 1))
                    if j == 1:
                        nc.vector.tensor_copy(
                            va_h[:, sc, :, 0:HD],
                            h[0][:, :GC].rearrange("p (h c) -> p h c", c=HD))
                return run
            # consumed by ctx(b0, sc) at slot 8 + sc//2; x block sc//4 paces
            nb, dl = sc, 8 + sc // 2
            return [(nb, dl, unit(j)) for j in range(2)]

        def ph3_unit(n, m):
            def run():
                if m == 0:
                    ot_ref[0] = ot_pool.tile([P, KO, QW], F32, tag="ot",
                                             name=f"ot{n}")
                g = ps_g.tile([P, QW], F32, tag="g", name="g")
                for c in range(2):
                    nc.tensor.matmul(g[:], wo_sb[:, c, m * P:(m + 1) * P],
                                     ct[:, c, n * QW:(n + 1) * QW],
                                     start=(c == 0), stop=(c == 1))
                nc.vector.tensor_copy(ot_ref[0][:, m, :], g[:])
                nc.sync.dma_start(
                    out_r[:, m, n * QW:(n + 1) * QW], ot_ref[0][:, m, :])
            return run

        trickle = []

        def trickle_add(nb, dl, fn):
            import bisect
            bisect.insort(trickle, (nb, dl, fn),
                          key=lambda u: u[1])

        def trickle_flush(g):
            while trickle and trickle[0][1] <= g:
                trickle.pop(0)[2]()

        def trickle_pace(g, budget=2, horizon=8):
            while (budget and trickle and trickle[0][0] <= g
                   and trickle[0][1] < g + horizon):
                trickle.pop(0)[2]()
                budget -= 1

        # build the schedule: QT(m,n) due at block (2n+m) slot 0; KT(m,j) due
        # at block m slot 4j (its k-chunk first read); V(sc) due before
        # ctx(b0, sc)
        sched = []
        for m in range(2):
            for n in range(NQ):
                due = (2 * n + m) * NKC
                sched += proj_units(wqs, qt, m, n, max(0, due - 10), due)
            for j in range(NQ):
                due = m * NKC + 4 * j
                sched += proj_units(wks, kt, m, j, max(0, due - 6), due)
        for sc in range(NKC):
            sched += v_units(sc)
        sched.sort(key=lambda u: u[1])
        trickle.extend(sched)

        # ---- fused phase 1+2+3 over one global score stream (blocks
        # b = n*2+hp, 16 kc each) ----
        exs = {}

        def normalize(c, n, hp):
            nsl = slice(n * QW, (n + 1) * QW)
            dn = dn_pool.tile([P, 4, QW], F32, tag="dn")
            bc = bc_pool.tile([P, 2, QW], F32, tag="bc")
            for e in range(2):
                # denom row: PSUM partition 64 -> SBUF partition 0
                nc.vector.tensor_copy(dn[0:1, e, :], c[64:65, e, :])
                nc.gpsimd.partition_broadcast(
                    dn[0:64, 2 + e, :], dn[0:1, e, :], channels=64)
                nc.vector.reciprocal_approx_fast(
                    bc[0:64, e, :], dn[0:64, 2 + e, :])
                nc.vector.tensor_tensor(
                    ct[e * 64:(e + 1) * 64, hp, nsl],
                    c[0:64, e, :], bc[0:64, e, :], mult)
            if dbg is not None and n == 0 and hp == 0:
                nc.sync.dma_start(dbg["dbg_dn"][:], dn[:])
                nc.sync.dma_start(dbg["dbg_bc"][:], bc[:])

        # block-local schedule: scores at slots 0..15, ctx catches up two per
        # slot at slots 8..15, so the next block's first ctx sits ~9 score
        # slots behind the previous normalize (covers its latency).
        CTX0 = NKC // 2
        for b in range(2 * NQ):
            n, hp = divmod(b, 2)
            nsl = slice(n * QW, (n + 1) * QW)
            c = ps_c.tile([P, 2, QW], F32, tag="c", name=f"c{b}")

            def ctx_mm(kc2):
                ex2 = exs.pop(kc2)
                for e in range(2):
                    h = 2 * hp + e
                    nc.tensor.matmul(
                        c[0:VW, e, :],
                        va[:, kc2, h * VW:(h + 1) * VW],
                        ex2[:, e, :],
                        start=(kc2 == 0), stop=(kc2 == NKC - 1))

            for kc in range(NKC):
                g = b * NKC + kc
                trickle_flush(g)
                sp = ps_s.tile([P, 2, QW], F32, tag="s")
                for e in range(2):
                    nc.tensor.matmul(
                        sp[:, e, :],
                        kt[e * 64:e * 64 + 64, hp, kc * P:(kc + 1) * P],
                        qt[e * 64:e * 64 + 64, hp, nsl],
                        start=True, stop=True)
                ex = ex_pool.tile([P, 2, QW], BF16, tag="ex")
                nc.scalar.activation(
                    ex[:].rearrange("p a b -> p (a b)"),
                    sp[:].rearrange("p a b -> p (a b)"),
                    mybir.ActivationFunctionType.Exp,
                    scale=0.125)
                exs[kc] = ex
                if dbg is not None and b == 0 and kc == 0:
                    nc.sync.dma_start(dbg["dbg_ex"][:], ex[:])
                trickle_pace(g)
                if kc >= CTX0:
                    ctx_mm(2 * (kc - CTX0))
                    ctx_mm(2 * (kc - CTX0) + 1)
            normalize(c, n, hp)
            if hp == 1:
                gp = (b + 1) * NKC
                for m in range(KO):
                    trickle_add(gp + 5 + m, gp + 9 + m, ph3_unit(n, m))

        for _, _, fn in trickle:
            fn()
        trickle.clear()

        if dbg is not None:
            nc.sync.dma_start(dbg["dbg_qt"][:], qt[:])
            nc.sync.dma_start(dbg["dbg_kt"][:], kt[:])
            nc.sync.dma_start(dbg["dbg_va"][:], va[:])
            nc.sync.dma_start(dbg["dbg_ct"][:], ct[:])


def _in_maps(x, wq_f, wk_f, wv_f, wo_f):
    maps = []
    for core in range(8):
        b, g = core // 4, core % 4
        cols = slice(g * GC, (g + 1) * GC)
        maps.append({
            "xt": np.ascontiguousarray(x[b].T).astype(NP_BF16),
            "wq": np.ascontiguousarray(wq_f[:, cols]).astype(NP_BF16),
            "wk": np.ascontiguousarray(wk_f[:, cols]).astype(NP_BF16),
            "wv": np.ascontiguousarray(wv_f[:, cols]).astype(NP_BF16),
            "wo": np.ascontiguousarray(wo_f[cols, :]).astype(NP_BF16),
        })
    return maps


def _prep(x, Wq, Wk, Wv, Wo, q_scale, k_scale, v_scale, o_scale):
    x = np.asarray(x, dtype=np.float32)
    wq_f = (np.asarray(Wq).T * np.asarray(q_scale).reshape(1, -1)).astype(np.float32)
    wk_f = (np.asarray(Wk).T * np.asarray(k_scale).reshape(1, -1)).astype(np.float32)
    wv_f = (np.asarray(Wv).T * np.asarray(v_scale).reshape(1, -1)).astype(np.float32)
    wo_f = (np.asarray(Wo).T * np.asarray(o_scale).reshape(1, -1)).astype(np.float32)
    return x, wq_f, wk_f, wv_f, wo_f


def run_traced(x, Wq, Wk, Wv, Wo, q_scale, k_scale, v_scale, o_scale):
    """Like kernel() but with NTFF tracing; returns (out, exec_time_ns, trace_path)."""
    x, wq_f, wk_f, wv_f, wo_f = _prep(x, Wq, Wk, Wv, Wo,
                                      q_scale, k_scale, v_scale, o_scale)
    nc = _build()
    res = run_bass_kernel_spmd(nc, _in_maps(x, wq_f, wk_f, wv_f, wo_f),
                               core_ids=list(range(8)), trace=True)
    out = np.zeros((x.shape[0], S, D), dtype=np.float32)
    for core in range(8):
        out[core // 4] += np.asarray(res.results[core]["out_t"],
                                     dtype=np.float32).T
    trace_path = None
    if res.instructions_and_trace is not None:
        trace_path = res.instructions_and_trace[1]
    return out, res.exec_time_ns, trace_path


def kernel(x, Wq, Wk, Wv, Wo, q_scale, k_scale, v_scale, o_scale):
    B = x.shape[0]
    x, wq_f, wk_f, wv_f, wo_f = _prep(x, Wq, Wk, Wv, Wo,
                                      q_scale, k_scale, v_scale, o_scale)
    nc = _build()
    res = run_bass_kernel_spmd(nc, _in_maps(x, wq_f, wk_f, wv_f, wo_f),
                               core_ids=list(range(8)))
    out = np.zeros((B, S, D), dtype=np.float32)
    for core in range(8):
        out[core // 4] += np.asarray(res.results[core]["out_t"],
                                     dtype=np.float32).T
    return out


# revision 35
# speedup vs baseline: 1.0370x; 1.0370x over previous
"""Multi-head self-attention (B=2, S=2048, D=1024, H=16) on 8 Trainium2 NeuronCores.

Sharding: batch x head-group. Core c = b*4 + g handles batch b and heads 4g..4g+3
(Megatron-style TP: Wq/Wk/Wv column-sharded, Wo row-sharded; partial outputs
summed on the host).

Per-core kernel layout ("T-layout": sequence on the free dim everywhere),
all matmul operands bf16, PSUM accumulation fp32:
  inputs (host-prepared):  xt [1024, 2048] = x[b].T;  wq/wk/wv [1024, 256]
  (scale-folded, transposed);  wo [256, 1024] (scale-folded, transposed)
  QT/KT = (w.T @ xt) [256, 2048]        d' on partitions, heads pair-stacked
  V     = (xt.T @ wv) [2048, 260]       natural layout + ones column per head
  scoresT[k, q] = KT_h-slices.T @ QT_h  per head, k on partitions (row-tiled
                                        T0/T8 pair: both heads of a pair run
                                        concurrently on the PE)
  expT = exp(scoresT / 8)               (no max subtraction: |scores| <~ 2)
  ctxT_aug[d+1, q] = [V_h | 1].T @ expT K=128 accumulation in one PSUM bank;
                                        row 64 = softmax denominator
  ctxT = ctxT_aug[0:64] * (1/denom)     recip on DVE, denom row broadcast via
                                        gpsimd partition_broadcast
  outT_partial = wo.T @ ctxT [1024, 2048]
Host: out[b] = sum_g outT[b, g].T

Pipeline structure: phase-2 score PSUM double-buffered so the Exp ACTIVATEs
(the critical path, ~128 x [128,1024]) stream back-to-back on the Scalar
engine while the PE interleaves scores/ctx with "filler" work (V projection,
remaining QT tiles, per-n output projection) to stay HAM-warm.
"""
import sys

sys.path.insert(0, "/opt/trn_rl_repo")

import numpy as np
import ml_dtypes

import concourse.bass as bass
import concourse.tile as tile
from concourse import bacc, mybir
from concourse.bass_utils import run_bass_kernel_spmd

F32 = mybir.dt.float32
BF16 = mybir.dt.bfloat16
NP_BF16 = ml_dtypes.bfloat16

S = 2048          # sequence length per batch
D = 1024          # embedding dim
HG = 4            # heads per core
HD = 64           # head dim
GC = HG * HD      # group cols = 256
P = 128
NQ = 4            # q chunks of 512
QW = 512          # q chunk width
NKC = 16          # key-position chunks of 128
KO = 8            # contraction chunks of 128 over D
VW = HD + 1       # V columns per head incl. ones column

_NC_CACHE = {}
DEBUG_DUMPS = False


def _build():
    if "nc" in _NC_CACHE:
        return _NC_CACHE["nc"]
    nc = bacc.Bacc(trn_type="TRN2", target_bir_lowering=False, debug=False)
    xt_d = nc.dram_tensor("xt", [D, S], BF16, kind="ExternalInput")
    wq_d = nc.dram_tensor("wq", [D, GC], BF16, kind="ExternalInput")
    wk_d = nc.dram_tensor("wk", [D, GC], BF16, kind="ExternalInput")
    wv_d = nc.dram_tensor("wv", [D, GC], BF16, kind="ExternalInput")
    wo_d = nc.dram_tensor("wo", [GC, D], BF16, kind="ExternalInput")
    out_d = nc.dram_tensor("out_t", [D, S], F32, kind="ExternalOutput")
    dbg = None
    if DEBUG_DUMPS:
        dbg = {
            "dbg_qt": nc.dram_tensor("dbg_qt", [P, 2, S], BF16,
                                     kind="ExternalOutput"),
            "dbg_kt": nc.dram_tensor("dbg_kt", [P, 2, S], BF16,
                                     kind="ExternalOutput"),
            "dbg_va": nc.dram_tensor("dbg_va", [P, NKC, HG * VW], BF16,
                                     kind="ExternalOutput"),
            "dbg_ct": nc.dram_tensor("dbg_ct", [P, 2, S], BF16,
                                     kind="ExternalOutput"),
            "dbg_dn": nc.dram_tensor("dbg_dn", [P, 4, QW], F32,
                                     kind="ExternalOutput"),
            "dbg_bc": nc.dram_tensor("dbg_bc", [P, 2, QW], F32,
                                     kind="ExternalOutput"),
            "dbg_ex": nc.dram_tensor("dbg_ex", [P, 2, QW], BF16,
                                     kind="ExternalOutput"),
        }
    with tile.TileContext(nc) as tc:
        _emit(nc, tc, xt_d, wq_d, wk_d, wv_d, wo_d, out_d, dbg)
    nc.compile()
    _NC_CACHE["nc"] = nc
    return nc


def _emit(nc, tc, xt_d, wq_d, wk_d, wv_d, wo_d, out_d, dbg=None):
    mult = mybir.AluOpType.mult
    with tc.tile_pool(name="big", bufs=1) as big, \
         tc.tile_pool(name="ex", bufs=10) as ex_pool, \
         tc.tile_pool(name="dn", bufs=2) as dn_pool, \
         tc.tile_pool(name="bcn", bufs=2) as bc_pool, \
         tc.tile_pool(name="ot", bufs=2) as ot_pool, \
         tc.tile_pool(name="ps_s", bufs=3, space="PSUM") as ps_s, \
         tc.tile_pool(name="ps_c", bufs=1, space="PSUM") as ps_c:

        # ---- persistent SBUF tensors ----
        xs = big.tile([P, KO, S], BF16)          # x.T  [d_in(128) x ko x s]
        wqs = big.tile([P, KO, GC], BF16)
        wks = big.tile([P, KO, GC], BF16)
        wvs = big.tile([P, KO, GC], BF16)
        wo_sb = big.tile([P, 2, D], BF16)        # [d'(128) x chunk x e]
        qt = big.tile([P, 2, S], BF16)           # head h at parts (h%2)*64, chunk h//2
        kt = big.tile([P, 2, S], BF16)
        va = big.tile([P, NKC, HG * VW], BF16)   # V natural + ones col per head
        ct = big.tile([P, 2, S], BF16)           # normalized ctxT, same layout as qt

        # ---- input DMAs: wq/x0/wk/wv get the wire first; x1-3/wo are held
        # back a few us (scheduler wait_until) so the first projections'
        # inputs land at full bandwidth, then the rest stream in just ahead
        # of their consumers ----
        xt_r = xt_d.rearrange("(ko p) s -> p ko s", p=P)
        nc.sync.dma_start(wqs[:], wq_d.rearrange("(ko p) m -> p ko m", p=P))
        nc.sync.dma_start(xs[:, :, 0:QW], xt_r[:, :, 0:QW])
        nc.sync.dma_start(wks[:], wk_d.rearrange("(ko p) m -> p ko m", p=P))
        nc.sync.dma_start(wvs[:], wv_d.rearrange("(ko p) m -> p ko m", p=P))
        for nn, ms in ((1, 0.005), (2, 0.007), (3, 0.009)):
            with tc.tile_wait_until(ms):
                nc.sync.dma_start(xs[:, :, nn * QW:(nn + 1) * QW],
                                  xt_r[:, :, nn * QW:(nn + 1) * QW])
        with tc.tile_wait_until(0.011):
            nc.sync.dma_start(wo_sb[:],
                              wo_d.rearrange("(c p) e -> p c e", p=P))

        # ones columns of V_aug (col HD of each VW-wide head block): bf16 1.0
        va_h = va[:].rearrange("p s (h c) -> p s h c", c=VW)
        for h in range(HG):
            nc.vector.memset(
                va_h[:, :, h, HD:HD + 1].bitcast(mybir.dt.uint16), 0x3F80)

        # ---- emission helpers (all big PSUM from the shared ps_s ring) ----
        def g_tile():
            g = ps_s.tile([P, 2, QW], F32, tag="s", name="g")
            return g

        def proj_tile(w_sb, dst, m, n):
            """QT/KT tile [128 x 512]: full K=128 contraction, single bank."""
            g = g_tile()
            for ko in range(KO):
                nc.tensor.matmul(g[:, 0, :], w_sb[:, ko, m * P:(m + 1) * P],
                                 xs[:, ko, n * QW:(n + 1) * QW],
                                 start=(ko == 0), stop=(ko == KO - 1))
            nc.vector.tensor_copy(dst[:, m, n * QW:(n + 1) * QW], g[:, 0, :])

        def v_tile(sc):
            """V natural tile for s-chunk sc: [128 x 256] into va."""
            g = g_tile()
            for ko in range(KO):
                nc.tensor.matmul(g[:, 0, :GC], xs[:, ko, sc * P:(sc + 1) * P],
                                 wvs[:, ko, :],
                                 start=(ko == 0), stop=(ko == KO - 1))
            nc.vector.tensor_copy(
                va_h[:, sc, :, 0:HD],
                g[:, 0, :GC].rearrange("p (h c) -> p h c", c=HD))

        ot_ref = [None]
        out_r = out_d.rearrange("(m p) q -> p m q", p=P)

        def ph3_mm(n, m):
            if m == 0:
                ot_ref[0] = ot_pool.tile([P, KO, QW], F32, tag="ot",
                                         name=f"ot{n}")
            g = g_tile()
            for c in range(2):
                nc.tensor.matmul(g[:, 0, :], wo_sb[:, c, m * P:(m + 1) * P],
                                 ct[:, c, n * QW:(n + 1) * QW],
                                 start=(c == 0), stop=(c == 1))
            nc.vector.tensor_copy(ot_ref[0][:, m, :], g[:, 0, :])
            nc.sync.dma_start(
                out_r[:, m, n * QW:(n + 1) * QW], ot_ref[0][:, m, :])

        # ---- fused phase 1+2+3: one global score stream (blocks b = n*2+hp,
        # 16 kc each), ctx stream lagging OFF behind so the next block's
        # scores always cover the normalize latency; KT/QT/V force-scheduled
        # into block 0; phase 3 rides the filler queue ----
        OFF = 6
        fillers = []
        cblocks = {}
        exs = {}

        def normalize(c, n, hp):
            nsl = slice(n * QW, (n + 1) * QW)
            dn = dn_pool.tile([P, 4, QW], F32, tag="dn")
            bc = bc_pool.tile([P, 2, QW], F32, tag="bc")
            for e in range(2):
                # denom row: PSUM partition 64 -> SBUF partition 0
                nc.vector.tensor_copy(dn[0:1, e, :], c[64:65, e, :])
                nc.gpsimd.partition_broadcast(
                    dn[0:64, 2 + e, :], dn[0:1, e, :], channels=64)
                nc.vector.reciprocal_approx_fast(
                    bc[0:64, e, :], dn[0:64, 2 + e, :])
                nc.vector.tensor_tensor(
                    ct[e * 64:(e + 1) * 64, hp, nsl],
                    c[0:64, e, :], bc[0:64, e, :], mult)
            if dbg is not None and n == 0 and hp == 0:
                nc.sync.dma_start(dbg["dbg_dn"][:], dn[:])
                nc.sync.dma_start(dbg["dbg_bc"][:], bc[:])

        def pre_extra(b, kc):
            """Work this slot's scores depend on (emitted before them)."""
            if b == 0:
                # interleave QT(n0)/KT(m0) into block 0 so the Scalar
                # engine starts on exp ~10us into the kernel
                if kc == 0:
                    proj_tile(wqs, qt, 0, 0)
                    proj_tile(wks, kt, 0, 0)
                elif kc % 4 == 0:
                    proj_tile(wks, kt, 0, kc // 4)
                return
            if b == 1 and kc % 4 == 0:
                proj_tile(wks, kt, 1, kc // 4)     # KT(m1) just-in-time

        def post_extra(b, kc):
            """Deferrable work (V tiles, QT for later blocks, ph3 pops)."""
            n, hp = divmod(b, 2)
            if b == 0:
                if kc == 1:
                    proj_tile(wqs, qt, 1, 0)
                v_tile(kc)              # va[kc] always precedes ctx[kc]
                return
            qt_slots = (3, 5) if b == 1 else (2, 4)
            if hp == 1 and n + 1 < NQ and kc in qt_slots:
                proj_tile(wqs, qt, qt_slots.index(kc), n + 1)
                return
            # ph3(n-1) pops: late slots of hp0 (normalize(n-1) has long
            # drained) and early slots of hp1
            pop_slots = (9, 11, 13, 15) if hp == 0 else (1, 3, 5, 7)
            if fillers and kc in pop_slots:
                fillers.pop(0)()

        # block-local schedule: scores at slots 0..15, ctx catches up two per
        # slot at slots 8..15, so the next block's first ctx sits ~9 score
        # slots behind the previous normalize (covers its latency).
        CTX0 = NKC // 2
        for b in range(2 * NQ):
            n, hp = divmod(b, 2)
            nsl = slice(n * QW, (n + 1) * QW)
            c = ps_c.tile([P, 2, QW], F32, tag="c", name=f"c{b}")

            def ctx_mm(kc2):
                ex2 = exs.pop(kc2)
                for e in range(2):
                    h = 2 * hp + e
                    nc.tensor.matmul(
                        c[0:VW, e, :],
                        va[:, kc2, h * VW:(h + 1) * VW],
                        ex2[:, e, :],
                        start=(kc2 == 0), stop=(kc2 == NKC - 1))

            for kc in range(NKC):
                pre_extra(b, kc)
                sp = ps_s.tile([P, 2, QW], F32, tag="s")
                for e in range(2):
                    nc.tensor.matmul(
                        sp[:, e, :],
                        kt[e * 64:e * 64 + 64, hp, kc * P:(kc + 1) * P],
                        qt[e * 64:e * 64 + 64, hp, nsl],
                        start=True, stop=True)
                ex = ex_pool.tile([P, 2, QW], BF16, tag="ex")
                nc.scalar.activation(
                    ex[:].rearrange("p a b -> p (a b)"),
                    sp[:].rearrange("p a b -> p (a b)"),
                    mybir.ActivationFunctionType.Exp,
                    scale=0.125)
                exs[kc] = ex
                if dbg is not None and b == 0 and kc == 0:
                    nc.sync.dma_start(dbg["dbg_ex"][:], ex[:])
                post_extra(b, kc)
                if kc >= CTX0:
                    ctx_mm(2 * (kc - CTX0))
                    ctx_mm(2 * (kc - CTX0) + 1)
            normalize(c, n, hp)
            if hp == 1:
                for m in range(KO):
                    fillers.append(lambda n=n, m=m: ph3_mm(n, m))

        while fillers:
            fillers.pop(0)()

        if dbg is not None:
            nc.sync.dma_start(dbg["dbg_qt"][:], qt[:])
            nc.sync.dma_start(dbg["dbg_kt"][:], kt[:])
            nc.sync.dma_start(dbg["dbg_va"][:], va[:])
            nc.sync.dma_start(dbg["dbg_ct"][:], ct[:])


def _in_maps(x, wq_f, wk_f, wv_f, wo_f):
    maps = []
    for core in range(8):
        b, g = core // 4, core % 4
        cols = slice(g * GC, (g + 1) * GC)
        maps.append({
            "xt": np.ascontiguousarray(x[b].T).astype(NP_BF16),
            "wq": np.ascontiguousarray(wq_f[:, cols]).astype(NP_BF16),
            "wk": np.ascontiguousarray(wk_f[:, cols]).astype(NP_BF16),
            "wv": np.ascontiguousarray(wv_f[:, cols]).astype(NP_BF16),
            "wo": np.ascontiguousarray(wo_f[cols, :]).astype(NP_BF16),
        })
    return maps


def _prep(x, Wq, Wk, Wv, Wo, q_scale, k_scale, v_scale, o_scale):
    x = np.asarray(x, dtype=np.float32)
    wq_f = (np.asarray(Wq).T * np.asarray(q_scale).reshape(1, -1)).astype(np.float32)
    wk_f = (np.asarray(Wk).T * np.asarray(k_scale).reshape(1, -1)).astype(np.float32)
    wv_f = (np.asarray(Wv).T * np.asarray(v_scale).reshape(1, -1)).astype(np.float32)
    wo_f = (np.asarray(Wo).T * np.asarray(o_scale).reshape(1, -1)).astype(np.float32)
    return x, wq_f, wk_f, wv_f, wo_f


def run_traced(x, Wq, Wk, Wv, Wo, q_scale, k_scale, v_scale, o_scale):
    """Like kernel() but with NTFF tracing; returns (out, exec_time_ns, trace_path)."""
    x, wq_f, wk_f, wv_f, wo_f = _prep(x, Wq, Wk, Wv, Wo,
                                      q_scale, k_scale, v_scale, o_scale)
    nc = _build()
    res = run_bass_kernel_spmd(nc, _in_maps(x, wq_f, wk_f, wv_f, wo_f),
                               core_ids=list(range(8)), trace=True)
    out = np.zeros((x.shape[0], S, D), dtype=np.float32)
    for core in range(8):
        out[core // 4] += np.asarray(res.results[core]["out_t"],
                                     dtype=np.float32).T
    trace_path = None
    if res.instructions_and_trace is not None:
        trace_path = res.instructions_and_trace[1]
    return out, res.exec_time_ns, trace_path


def kernel(x, Wq, Wk, Wv, Wo, q_scale, k_scale, v_scale, o_scale):
    B = x.shape[0]
    x, wq_f, wk_f, wv_f, wo_f = _prep(x, Wq, Wk, Wv, Wo,
                                      q_scale, k_scale, v_scale, o_scale)
    nc = _build()
    res = run_bass_kernel_spmd(nc, _in_maps(x, wq_f, wk_f, wv_f, wo_f),
                               core_ids=list(range(8)))
    out = np.zeros((B, S, D), dtype=np.float32)
    for core in range(8):
        out[core // 4] += np.asarray(res.results[core]["out_t"],
                                     dtype=np.float32).T
    return out


# revision 39
# speedup vs baseline: 1.0380x; 1.0009x over previous
"""Multi-head self-attention (B=2, S=2048, D=1024, H=16) on 8 Trainium2 NeuronCores.

Sharding: batch x head-group. Core c = b*4 + g handles batch b and heads 4g..4g+3
(Megatron-style TP: Wq/Wk/Wv column-sharded, Wo row-sharded; partial outputs
summed on the host).

Per-core kernel layout ("T-layout": sequence on the free dim everywhere),
all matmul operands bf16, PSUM accumulation fp32:
  inputs (host-prepared):  xt [1024, 2048] = x[b].T;  wq/wk/wv [1024, 256]
  (scale-folded, transposed);  wo [256, 1024] (scale-folded, transposed)
  QT/KT = (w.T @ xt) [256, 2048]        d' on partitions, heads pair-stacked
  V     = (xt.T @ wv) [2048, 260]       natural layout + ones column per head
  scoresT[k, q] = KT_h-slices.T @ QT_h  per head, k on partitions (row-tiled
                                        T0/T8 pair: both heads of a pair run
                                        concurrently on the PE)
  expT = exp(scoresT / 8)               (no max subtraction: |scores| <~ 2)
  ctxT_aug[d+1, q] = [V_h | 1].T @ expT K=128 accumulation in one PSUM bank;
                                        row 64 = softmax denominator
  ctxT = ctxT_aug[0:64] * (1/denom)     recip on DVE, denom row broadcast via
                                        gpsimd partition_broadcast
  outT_partial = wo.T @ ctxT [1024, 2048]
Host: out[b] = sum_g outT[b, g].T

Pipeline structure: phase-2 score PSUM double-buffered so the Exp ACTIVATEs
(the critical path, ~128 x [128,1024]) stream back-to-back on the Scalar
engine while the PE interleaves scores/ctx with "filler" work (V projection,
remaining QT tiles, per-n output projection) to stay HAM-warm.
"""
import sys

sys.path.insert(0, "/opt/trn_rl_repo")

import numpy as np
import ml_dtypes

import concourse.bass as bass
import concourse.tile as tile
from concourse import bacc, mybir
from concourse.bass_utils import run_bass_kernel_spmd

F32 = mybir.dt.float32
BF16 = mybir.dt.bfloat16
NP_BF16 = ml_dtypes.bfloat16

S = 2048          # sequence length per batch
D = 1024          # embedding dim
HG = 4            # heads per core
HD = 64           # head dim
GC = HG * HD      # group cols = 256
P = 128
NQ = 4            # q chunks of 512
QW = 512          # q chunk width
NKC = 16          # key-position chunks of 128
KO = 8            # contraction chunks of 128 over D
VW = HD + 1       # V columns per head incl. ones column

_NC_CACHE = {}
DEBUG_DUMPS = False


def _build():
    if "nc" in _NC_CACHE:
        return _NC_CACHE["nc"]
    nc = bacc.Bacc(trn_type="TRN2", target_bir_lowering=False, debug=False)
    xt_d = nc.dram_tensor("xt", [D, S], BF16, kind="ExternalInput")
    wq_d = nc.dram_tensor("wq", [D, GC], BF16, kind="ExternalInput")
    wk_d = nc.dram_tensor("wk", [D, GC], BF16, kind="ExternalInput")
    wv_d = nc.dram_tensor("wv", [D, GC], BF16, kind="ExternalInput")
    wo_d = nc.dram_tensor("wo", [GC, D], BF16, kind="ExternalInput")
    out_d = nc.dram_tensor("out_t", [D, S], F32, kind="ExternalOutput")
    dbg = None
    if DEBUG_DUMPS:
        dbg = {
            "dbg_qt": nc.dram_tensor("dbg_qt", [P, 2, S], BF16,
                                     kind="ExternalOutput"),
            "dbg_kt": nc.dram_tensor("dbg_kt", [P, 2, S], BF16,
                                     kind="ExternalOutput"),
            "dbg_va": nc.dram_tensor("dbg_va", [P, NKC, HG * VW], BF16,
                                     kind="ExternalOutput"),
            "dbg_ct": nc.dram_tensor("dbg_ct", [P, 2, S], BF16,
                                     kind="ExternalOutput"),
            "dbg_dn": nc.dram_tensor("dbg_dn", [P, 4, QW], F32,
                                     kind="ExternalOutput"),
            "dbg_bc": nc.dram_tensor("dbg_bc", [P, 2, QW], F32,
                                     kind="ExternalOutput"),
            "dbg_ex": nc.dram_tensor("dbg_ex", [P, 2, QW], BF16,
                                     kind="ExternalOutput"),
        }
    with tile.TileContext(nc) as tc:
        _emit(nc, tc, xt_d, wq_d, wk_d, wv_d, wo_d, out_d, dbg)
    nc.compile()
    _NC_CACHE["nc"] = nc
    return nc


def _emit(nc, tc, xt_d, wq_d, wk_d, wv_d, wo_d, out_d, dbg=None):
    mult = mybir.AluOpType.mult
    with tc.tile_pool(name="big", bufs=1) as big, \
         tc.tile_pool(name="ex", bufs=10) as ex_pool, \
         tc.tile_pool(name="dn", bufs=2) as dn_pool, \
         tc.tile_pool(name="bcn", bufs=2) as bc_pool, \
         tc.tile_pool(name="ot", bufs=2) as ot_pool, \
         tc.tile_pool(name="ps_s", bufs=3, space="PSUM") as ps_s, \
         tc.tile_pool(name="ps_c", bufs=1, space="PSUM") as ps_c:

        # ---- persistent SBUF tensors ----
        xs = big.tile([P, KO, S], BF16)          # x.T  [d_in(128) x ko x s]
        wqs = big.tile([P, KO, GC], BF16)
        wks = big.tile([P, KO, GC], BF16)
        wvs = big.tile([P, KO, GC], BF16)
        wo_sb = big.tile([P, 2, D], BF16)        # [d'(128) x chunk x e]
        qt = big.tile([P, 2, S], BF16)           # head h at parts (h%2)*64, chunk h//2
        kt = big.tile([P, 2, S], BF16)
        va = big.tile([P, NKC, HG * VW], BF16)   # V natural + ones col per head
        ct = big.tile([P, 2, S], BF16)           # normalized ctxT, same layout as qt

        # ---- input DMAs (all upfront; transfers share DMA bandwidth and
        # finish ~17us in, which matches the PE's phase-1 warm-up anyway) ----
        xt_r = xt_d.rearrange("(ko p) s -> p ko s", p=P)
        nc.sync.dma_start(wqs[:], wq_d.rearrange("(ko p) m -> p ko m", p=P))
        nc.sync.dma_start(xs[:, :, 0:QW], xt_r[:, :, 0:QW])
        nc.sync.dma_start(wks[:], wk_d.rearrange("(ko p) m -> p ko m", p=P))
        nc.sync.dma_start(wvs[:], wv_d.rearrange("(ko p) m -> p ko m", p=P))
        for nn in range(1, NQ):
            nc.sync.dma_start(xs[:, :, nn * QW:(nn + 1) * QW],
                              xt_r[:, :, nn * QW:(nn + 1) * QW])
        nc.sync.dma_start(wo_sb[:], wo_d.rearrange("(c p) e -> p c e", p=P))

        # ones columns of V_aug (col HD of each VW-wide head block): bf16 1.0
        va_h = va[:].rearrange("p s (h c) -> p s h c", c=VW)
        for h in range(HG):
            nc.vector.memset(
                va_h[:, :, h, HD:HD + 1].bitcast(mybir.dt.uint16), 0x3F80)

        # ---- emission helpers (all big PSUM from the shared ps_s ring) ----
        def g_tile():
            g = ps_s.tile([P, 2, QW], F32, tag="s", name="g")
            return g

        def proj_tile(w_sb, dst, m, n):
            """QT/KT tile [128 x 512]: full K=128 contraction, single bank."""
            g = g_tile()
            for ko in range(KO):
                nc.tensor.matmul(g[:, 0, :], w_sb[:, ko, m * P:(m + 1) * P],
                                 xs[:, ko, n * QW:(n + 1) * QW],
                                 start=(ko == 0), stop=(ko == KO - 1))
            nc.vector.tensor_copy(dst[:, m, n * QW:(n + 1) * QW], g[:, 0, :])

        def v_tile(sc):
            """V natural tile for s-chunk sc: [128 x 256] into va."""
            g = g_tile()
            for ko in range(KO):
                nc.tensor.matmul(g[:, 0, :GC], xs[:, ko, sc * P:(sc + 1) * P],
                                 wvs[:, ko, :],
                                 start=(ko == 0), stop=(ko == KO - 1))
            nc.vector.tensor_copy(
                va_h[:, sc, :, 0:HD],
                g[:, 0, :GC].rearrange("p (h c) -> p h c", c=HD))

        ot_ref = [None]
        out_r = out_d.rearrange("(m p) q -> p m q", p=P)

        def ph3_mm(n, m):
            if m == 0:
                ot_ref[0] = ot_pool.tile([P, KO, QW], F32, tag="ot",
                                         name=f"ot{n}")
            g = g_tile()
            for c in range(2):
                nc.tensor.matmul(g[:, 0, :], wo_sb[:, c, m * P:(m + 1) * P],
                                 ct[:, c, n * QW:(n + 1) * QW],
                                 start=(c == 0), stop=(c == 1))
            nc.vector.tensor_copy(ot_ref[0][:, m, :], g[:, 0, :])
            nc.sync.dma_start(
                out_r[:, m, n * QW:(n + 1) * QW], ot_ref[0][:, m, :])

        # ---- fused phase 1+2+3: one global score stream (blocks b = n*2+hp,
        # 16 kc each), ctx stream lagging OFF behind so the next block's
        # scores always cover the normalize latency; KT/QT/V force-scheduled
        # into block 0; phase 3 rides the filler queue ----
        OFF = 6
        fillers = []
        cblocks = {}
        exs = {}

        def normalize(c, n, hp):
            nsl = slice(n * QW, (n + 1) * QW)
            dn = dn_pool.tile([P, 4, QW], F32, tag="dn")
            bc = bc_pool.tile([P, 2, QW], F32, tag="bc")
            for e in range(2):
                # denom row: PSUM partition 64 -> SBUF partition 0
                nc.vector.tensor_copy(dn[0:1, e, :], c[64:65, e, :])
                nc.gpsimd.partition_broadcast(
                    dn[0:64, 2 + e, :], dn[0:1, e, :], channels=64)
                nc.vector.reciprocal_approx_fast(
                    bc[0:64, e, :], dn[0:64, 2 + e, :])
                nc.vector.tensor_tensor(
                    ct[e * 64:(e + 1) * 64, hp, nsl],
                    c[0:64, e, :], bc[0:64, e, :], mult)
            if dbg is not None and n == 0 and hp == 0:
                nc.sync.dma_start(dbg["dbg_dn"][:], dn[:])
                nc.sync.dma_start(dbg["dbg_bc"][:], bc[:])

        def pre_extra(b, kc):
            """Work this slot's scores depend on (emitted before them)."""
            if b == 0:
                # interleave QT(n0)/KT(m0) into block 0 so the Scalar
                # engine starts on exp ~10us into the kernel
                if kc == 0:
                    proj_tile(wqs, qt, 0, 0)
                    proj_tile(wks, kt, 0, 0)
                elif kc % 4 == 0:
                    proj_tile(wks, kt, 0, kc // 4)
                return
            if b == 1 and kc % 4 == 0:
                proj_tile(wks, kt, 1, kc // 4)     # KT(m1) just-in-time

        def post_extra(b, kc):
            """Deferrable work (V tiles, QT for later blocks, ph3 pops)."""
            n, hp = divmod(b, 2)
            if b == 0:
                if kc == 1:
                    proj_tile(wqs, qt, 1, 0)
                v_tile(kc)              # va[kc] always precedes ctx[kc]
                return
            qt_slots = (3, 5) if b == 1 else (2, 4)
            if hp == 1 and n + 1 < NQ and kc in qt_slots:
                proj_tile(wqs, qt, qt_slots.index(kc), n + 1)
                return
            # ph3(n-1) pops: late slots of hp0 (normalize(n-1) has long
            # drained) and early slots of hp1
            pop_slots = (9, 11, 13, 15) if hp == 0 else (1, 3, 5, 7)
            if fillers and kc in pop_slots:
                fillers.pop(0)()

        # block-local schedule: scores at slots 0..15, ctx catches up two per
        # slot at slots 8..15, so the next block's first ctx sits ~9 score
        # slots behind the previous normalize (covers its latency).
        CTX0 = NKC // 2
        for b in range(2 * NQ):
            n, hp = divmod(b, 2)
            nsl = slice(n * QW, (n + 1) * QW)
            c = ps_c.tile([P, 2, QW], F32, tag="c", name=f"c{b}")

            def ctx_mm(kc2):
                ex2 = exs.pop(kc2)
                for e in range(2):
                    h = 2 * hp + e
                    nc.tensor.matmul(
                        c[0:VW, e, :],
                        va[:, kc2, h * VW:(h + 1) * VW],
                        ex2[:, e, :],
                        start=(kc2 == 0), stop=(kc2 == NKC - 1))

            for kc in range(NKC):
                pre_extra(b, kc)
                sp = ps_s.tile([P, 2, QW], F32, tag="s")
                for e in range(2):
                    nc.tensor.matmul(
                        sp[:, e, :],
                        kt[e * 64:e * 64 + 64, hp, kc * P:(kc + 1) * P],
                        qt[e * 64:e * 64 + 64, hp, nsl],
                        start=True, stop=True)
                ex = ex_pool.tile([P, 2, QW], BF16, tag="ex")
                nc.scalar.activation(
                    ex[:].rearrange("p a b -> p (a b)"),
                    sp[:].rearrange("p a b -> p (a b)"),
                    mybir.ActivationFunctionType.Exp,
                    scale=0.125)
                exs[kc] = ex
                if dbg is not None and b == 0 and kc == 0:
                    nc.sync.dma_start(dbg["dbg_ex"][:], ex[:])
                post_extra(b, kc)
                if kc >= CTX0:
                    ctx_mm(2 * (kc - CTX0))
                    ctx_mm(2 * (kc - CTX0) + 1)
            normalize(c, n, hp)
            if hp == 1:
                for m in range(KO):
                    fillers.append(lambda n=n, m=m: ph3_mm(n, m))

        while fillers:
            fillers.pop(0)()

        if dbg is not None:
            nc.sync.dma_start(dbg["dbg_qt"][:], qt[:])
            nc.sync.dma_start(dbg["dbg_kt"][:], kt[:])
            nc.sync.dma_start(dbg["dbg_va"][:], va[:])
            nc.sync.dma_start(dbg["dbg_ct"][:], ct[:])


def _in_maps(x, wq_f, wk_f, wv_f, wo_f):
    maps = []
    for core in range(8):
        b, g = core // 4, core % 4
        cols = slice(g * GC, (g + 1) * GC)
        maps.append({
            "xt": np.ascontiguousarray(x[b].T).astype(NP_BF16),
            "wq": np.ascontiguousarray(wq_f[:, cols]).astype(NP_BF16),
            "wk": np.ascontiguousarray(wk_f[:, cols]).astype(NP_BF16),
            "wv": np.ascontiguousarray(wv_f[:, cols]).astype(NP_BF16),
            "wo": np.ascontiguousarray(wo_f[cols, :]).astype(NP_BF16),
        })
    return maps


def _prep(x, Wq, Wk, Wv, Wo, q_scale, k_scale, v_scale, o_scale):
    x = np.asarray(x, dtype=np.float32)
    wq_f = (np.asarray(Wq).T * np.asarray(q_scale).reshape(1, -1)).astype(np.float32)
    wk_f = (np.asarray(Wk).T * np.asarray(k_scale).reshape(1, -1)).astype(np.float32)
    wv_f = (np.asarray(Wv).T * np.asarray(v_scale).reshape(1, -1)).astype(np.float32)
    wo_f = (np.asarray(Wo).T * np.asarray(o_scale).reshape(1, -1)).astype(np.float32)
    return x, wq_f, wk_f, wv_f, wo_f


def run_traced(x, Wq, Wk, Wv, Wo, q_scale, k_scale, v_scale, o_scale):
    """Like kernel() but with NTFF tracing; returns (out, exec_time_ns, trace_path)."""
    x, wq_f, wk_f, wv_f, wo_f = _prep(x, Wq, Wk, Wv, Wo,
                                      q_scale, k_scale, v_scale, o_scale)
    nc = _build()
    res = run_bass_kernel_spmd(nc, _in_maps(x, wq_f, wk_f, wv_f, wo_f),
                               core_ids=list(range(8)), trace=True)
    out = np.zeros((x.shape[0], S, D), dtype=np.float32)
    for core in range(8):
        out[core // 4] += np.asarray(res.results[core]["out_t"],
                                     dtype=np.float32).T
    trace_path = None
    if res.instructions_and_trace is not None:
        trace_path = res.instructions_and_trace[1]
    return out, res.exec_time_ns, trace_path


def kernel(x, Wq, Wk, Wv, Wo, q_scale, k_scale, v_scale, o_scale):
    B = x.shape[0]
    x, wq_f, wk_f, wv_f, wo_f = _prep(x, Wq, Wk, Wv, Wo,
                                      q_scale, k_scale, v_scale, o_scale)
    nc = _build()
    res = run_bass_kernel_spmd(nc, _in_maps(x, wq_f, wk_f, wv_f, wo_f),
                               core_ids=list(range(8)))
    out = np.zeros((B, S, D), dtype=np.float32)
    for core in range(8):
        out[core // 4] += np.asarray(res.results[core]["out_t"],
                                     dtype=np.float32).T
    return out


# revision 41
# speedup vs baseline: 1.0530x; 1.0145x over previous
"""Multi-head self-attention (B=2, S=2048, D=1024, H=16) on 8 Trainium2 NeuronCores.

Sharding: batch x head-group. Core c = b*4 + g handles batch b and heads 4g..4g+3
(Megatron-style TP: Wq/Wk/Wv column-sharded, Wo row-sharded; partial outputs
summed on the host).

Per-core kernel layout ("T-layout": sequence on the free dim everywhere),
all matmul operands bf16, PSUM accumulation fp32:
  inputs (host-prepared):  xt [1024, 2048] = x[b].T;  wq/wk/wv [1024, 256]
  (scale-folded, transposed);  wo [256, 1024] (scale-folded, transposed)
  QT/KT = (w.T @ xt) [256, 2048]        d' on partitions, heads pair-stacked
  V     = (xt.T @ wv) [2048, 260]       natural layout + ones column per head
  scoresT[k, q] = KT_h-slices.T @ QT_h  per head, k on partitions (row-tiled
                                        T0/T8 pair: both heads of a pair run
                                        concurrently on the PE)
  expT = exp(scoresT / 8)               (no max subtraction: |scores| <~ 2)
  ctxT_aug[d+1, q] = [V_h | 1].T @ expT K=128 accumulation in one PSUM bank;
                                        row 64 = softmax denominator
  ctxT = ctxT_aug[0:64] * (1/denom)     recip on DVE, denom row broadcast via
                                        gpsimd partition_broadcast
  outT_partial = wo.T @ ctxT [1024, 2048]
Host: out[b] = sum_g outT[b, g].T

Pipeline structure: phase-2 score PSUM double-buffered so the Exp ACTIVATEs
(the critical path, ~128 x [128,1024]) stream back-to-back on the Scalar
engine while the PE interleaves scores/ctx with "filler" work (V projection,
remaining QT tiles, per-n output projection) to stay HAM-warm.
"""
import sys

sys.path.insert(0, "/opt/trn_rl_repo")

import numpy as np
import ml_dtypes

import concourse.bass as bass
import concourse.tile as tile
from concourse import bacc, mybir
from concourse.bass_utils import run_bass_kernel_spmd

F32 = mybir.dt.float32
BF16 = mybir.dt.bfloat16
NP_BF16 = ml_dtypes.bfloat16

S = 2048          # sequence length per batch
D = 1024          # embedding dim
HG = 4            # heads per core
HD = 64           # head dim
GC = HG * HD      # group cols = 256
P = 128
NQ = 4            # q chunks of 512
QW = 512          # q chunk width
NKC = 16          # key-position chunks of 128
KO = 8            # contraction chunks of 128 over D
VW = HD + 1       # V columns per head incl. ones column

_NC_CACHE = {}
DEBUG_DUMPS = False


def _build():
    if "nc" in _NC_CACHE:
        return _NC_CACHE["nc"]
    nc = bacc.Bacc(trn_type="TRN2", target_bir_lowering=False, debug=False)
    xt_d = nc.dram_tensor("xt", [D, S], BF16, kind="ExternalInput")
    wq_d = nc.dram_tensor("wq", [D, GC], BF16, kind="ExternalInput")
    wk_d = nc.dram_tensor("wk", [D, GC], BF16, kind="ExternalInput")
    wv_d = nc.dram_tensor("wv", [D, GC], BF16, kind="ExternalInput")
    wo_d = nc.dram_tensor("wo", [GC, D], BF16, kind="ExternalInput")
    out_d = nc.dram_tensor("out_t", [D, S], F32, kind="ExternalOutput")
    dbg = None
    if DEBUG_DUMPS:
        dbg = {
            "dbg_qt": nc.dram_tensor("dbg_qt", [P, 2, S], BF16,
                                     kind="ExternalOutput"),
            "dbg_kt": nc.dram_tensor("dbg_kt", [P, 2, S], BF16,
                                     kind="ExternalOutput"),
            "dbg_va": nc.dram_tensor("dbg_va", [P, NKC, HG * VW], BF16,
                                     kind="ExternalOutput"),
            "dbg_ct": nc.dram_tensor("dbg_ct", [P, 2, S], BF16,
                                     kind="ExternalOutput"),
            "dbg_dn": nc.dram_tensor("dbg_dn", [P, 4, QW], F32,
                                     kind="ExternalOutput"),
            "dbg_bc": nc.dram_tensor("dbg_bc", [P, 2, QW], F32,
                                     kind="ExternalOutput"),
            "dbg_ex": nc.dram_tensor("dbg_ex", [P, 2, QW], BF16,
                                     kind="ExternalOutput"),
        }
    with tile.TileContext(nc) as tc:
        _emit(nc, tc, xt_d, wq_d, wk_d, wv_d, wo_d, out_d, dbg)
    nc.compile()
    _NC_CACHE["nc"] = nc
    return nc


def _emit(nc, tc, xt_d, wq_d, wk_d, wv_d, wo_d, out_d, dbg=None):
    mult = mybir.AluOpType.mult
    with tc.tile_pool(name="big", bufs=1) as big, \
         tc.tile_pool(name="ex", bufs=10) as ex_pool, \
         tc.tile_pool(name="dn", bufs=2) as dn_pool, \
         tc.tile_pool(name="bcn", bufs=2) as bc_pool, \
         tc.tile_pool(name="ot", bufs=2) as ot_pool, \
         tc.tile_pool(name="ps_s", bufs=3, space="PSUM") as ps_s, \
         tc.tile_pool(name="ps_c", bufs=1, space="PSUM") as ps_c:

        # ---- persistent SBUF tensors ----
        xs = big.tile([P, KO, S], BF16)          # x.T  [d_in(128) x ko x s]
        wqs = big.tile([P, KO, GC], BF16)
        wks = big.tile([P, KO, GC], BF16)
        wvs = big.tile([P, KO, GC], BF16)
        wo_sb = big.tile([P, 2, D], BF16)        # [d'(128) x chunk x e]
        qt = big.tile([P, 2, S], BF16)           # head h at parts (h%2)*64, chunk h//2
        kt = big.tile([P, 2, S], BF16)
        va = big.tile([P, NKC, HG * VW], BF16)   # V natural + ones col per head
        ct = big.tile([P, 2, S], BF16)           # normalized ctxT, same layout as qt

        # ---- input DMAs (all upfront; transfers share DMA bandwidth and
        # finish ~17us in, which matches the PE's phase-1 warm-up anyway) ----
        xt_r = xt_d.rearrange("(ko p) s -> p ko s", p=P)
        nc.sync.dma_start(wqs[:], wq_d.rearrange("(ko p) m -> p ko m", p=P))
        nc.sync.dma_start(xs[:, :, 0:QW], xt_r[:, :, 0:QW])
        nc.sync.dma_start(wks[:], wk_d.rearrange("(ko p) m -> p ko m", p=P))
        nc.sync.dma_start(wvs[:], wv_d.rearrange("(ko p) m -> p ko m", p=P))
        for nn in range(1, NQ):
            nc.sync.dma_start(xs[:, :, nn * QW:(nn + 1) * QW],
                              xt_r[:, :, nn * QW:(nn + 1) * QW])
        nc.sync.dma_start(wo_sb[:], wo_d.rearrange("(c p) e -> p c e", p=P))

        # ones columns of V_aug (col HD of each VW-wide head block): bf16 1.0
        va_h = va[:].rearrange("p s (h c) -> p s h c", c=VW)
        for h in range(HG):
            nc.vector.memset(
                va_h[:, :, h, HD:HD + 1].bitcast(mybir.dt.uint16), 0x3F80)

        # ---- emission helpers (all big PSUM from the shared ps_s ring) ----
        def g_tile():
            g = ps_s.tile([P, 2, QW], F32, tag="s", name="g")
            return g

        def proj_tile(w_sb, dst, m, n):
            """QT/KT tile [128 x 512]: full K=128 contraction, single bank."""
            g = g_tile()
            for ko in range(KO):
                nc.tensor.matmul(g[:, 0, :], w_sb[:, ko, m * P:(m + 1) * P],
                                 xs[:, ko, n * QW:(n + 1) * QW],
                                 start=(ko == 0), stop=(ko == KO - 1))
            nc.vector.tensor_copy(dst[:, m, n * QW:(n + 1) * QW], g[:, 0, :])

        def v_tile(sc):
            """V natural tile for s-chunk sc: [128 x 256] into va."""
            g = g_tile()
            for ko in range(KO):
                nc.tensor.matmul(g[:, 0, :GC], xs[:, ko, sc * P:(sc + 1) * P],
                                 wvs[:, ko, :],
                                 start=(ko == 0), stop=(ko == KO - 1))
            nc.vector.tensor_copy(
                va_h[:, sc, :, 0:HD],
                g[:, 0, :GC].rearrange("p (h c) -> p h c", c=HD))

        ot_ref = [None]
        out_r = out_d.rearrange("(m p) q -> p m q", p=P)

        def ph3_mm(n, m):
            if m == 0:
                ot_ref[0] = ot_pool.tile([P, KO, QW], F32, tag="ot",
                                         name=f"ot{n}")
            g = g_tile()
            for c in range(2):
                nc.tensor.matmul(g[:, 0, :], wo_sb[:, c, m * P:(m + 1) * P],
                                 ct[:, c, n * QW:(n + 1) * QW],
                                 start=(c == 0), stop=(c == 1))
            nc.vector.tensor_copy(ot_ref[0][:, m, :], g[:, 0, :])
            nc.sync.dma_start(
                out_r[:, m, n * QW:(n + 1) * QW], ot_ref[0][:, m, :])

        # ---- fused phase 1+2+3: one global score stream (blocks b = n*2+hp,
        # 16 kc each), ctx stream lagging OFF behind so the next block's
        # scores always cover the normalize latency; KT/QT/V force-scheduled
        # into block 0; phase 3 rides the filler queue ----
        OFF = 6
        fillers = []
        cblocks = {}
        exs = {}

        def normalize(c, n, hp):
            nsl = slice(n * QW, (n + 1) * QW)
            dn = dn_pool.tile([P, 4, QW], F32, tag="dn")
            bc = bc_pool.tile([P, 2, QW], F32, tag="bc")
            # phase-interleaved so the two heads' chains pipeline across
            # DVE and GpSimd instead of running back-to-back
            for e in range(2):
                # denom row: PSUM partition 64 -> SBUF partition 0
                nc.vector.tensor_copy(dn[0:1, e, :], c[64:65, e, :])
            for e in range(2):
                nc.gpsimd.partition_broadcast(
                    dn[0:64, 2 + e, :], dn[0:1, e, :], channels=64)
            for e in range(2):
                nc.vector.reciprocal_approx_fast(
                    bc[0:64, e, :], dn[0:64, 2 + e, :])
            for e in range(2):
                nc.vector.tensor_tensor(
                    ct[e * 64:(e + 1) * 64, hp, nsl],
                    c[0:64, e, :], bc[0:64, e, :], mult)
            if dbg is not None and n == 0 and hp == 0:
                nc.sync.dma_start(dbg["dbg_dn"][:], dn[:])
                nc.sync.dma_start(dbg["dbg_bc"][:], bc[:])

        def pre_extra(b, kc):
            """Work this slot's scores depend on (emitted before them)."""
            if b == 0:
                # interleave QT(n0)/KT(m0) into block 0 so the Scalar
                # engine starts on exp ~10us into the kernel
                if kc == 0:
                    proj_tile(wqs, qt, 0, 0)
                    proj_tile(wks, kt, 0, 0)
                elif kc % 4 == 0:
                    proj_tile(wks, kt, 0, kc // 4)
                return
            if b == 1 and kc % 4 == 0:
                proj_tile(wks, kt, 1, kc // 4)     # KT(m1) just-in-time

        def post_extra(b, kc):
            """Deferrable work (V tiles, QT for later blocks, ph3 pops)."""
            n, hp = divmod(b, 2)
            if b == 0:
                if kc == 1:
                    proj_tile(wqs, qt, 1, 0)
                v_tile(kc)              # va[kc] always precedes ctx[kc]
                return
            qt_slots = (3, 5) if b == 1 else (2, 4)
            if hp == 1 and n + 1 < NQ and kc in qt_slots:
                proj_tile(wqs, qt, qt_slots.index(kc), n + 1)
                return
            # ph3(n-1) pops: late slots of hp0 (normalize(n-1) has long
            # drained) and early slots of hp1
            pop_slots = (9, 11, 13, 15) if hp == 0 else (1, 3, 5, 7)
            if fillers and kc in pop_slots:
                fillers.pop(0)()

        # block-local schedule: scores at slots 0..15, ctx catches up two per
        # slot at slots 8..15, so the next block's first ctx sits ~9 score
        # slots behind the previous normalize (covers its latency).
        CTX0 = NKC // 2
        for b in range(2 * NQ):
            n, hp = divmod(b, 2)
            nsl = slice(n * QW, (n + 1) * QW)
            c = ps_c.tile([P, 2, QW], F32, tag="c", name=f"c{b}")

            def ctx_mm(kc2):
                ex2 = exs.pop(kc2)
                for e in range(2):
                    h = 2 * hp + e
                    nc.tensor.matmul(
                        c[0:VW, e, :],
                        va[:, kc2, h * VW:(h + 1) * VW],
                        ex2[:, e, :],
                        start=(kc2 == 0), stop=(kc2 == NKC - 1))

            for kc in range(NKC):
                pre_extra(b, kc)
                sp = ps_s.tile([P, 2, QW], F32, tag="s")
                for e in range(2):
                    nc.tensor.matmul(
                        sp[:, e, :],
                        kt[e * 64:e * 64 + 64, hp, kc * P:(kc + 1) * P],
                        qt[e * 64:e * 64 + 64, hp, nsl],
                        start=True, stop=True)
                ex = ex_pool.tile([P, 2, QW], BF16, tag="ex")
                nc.scalar.activation(
                    ex[:].rearrange("p a b -> p (a b)"),
                    sp[:].rearrange("p a b -> p (a b)"),
                    mybir.ActivationFunctionType.Exp,
                    scale=0.125)
                exs[kc] = ex
                if dbg is not None and b == 0 and kc == 0:
                    nc.sync.dma_start(dbg["dbg_ex"][:], ex[:])
                post_extra(b, kc)
                if kc >= CTX0:
                    ctx_mm(2 * (kc - CTX0))
                    ctx_mm(2 * (kc - CTX0) + 1)
            normalize(c, n, hp)
            if hp == 1:
                for m in range(KO):
                    fillers.append(lambda n=n, m=m: ph3_mm(n, m))

        # warm-keepers: dependency-anchored (read ct chunk 0 of n3, ready
        # since the second-to-last normalize) throwaway matmuls that keep the
        # PE's HAM clock-gate at 2.4GHz through the final normalize wait, so
        # the trailing phase-3 tiles don't run at 1.2GHz. Results unread.
        warm = ps_s.tile([P, 2, QW], F32, tag="s", name="warm")
        for w in range(10):
            nc.tensor.matmul(
                warm[:, 0, :], wo_sb[:, 0, 0:P],
                ct[:, 0, (NQ - 1) * QW:NQ * QW],
                start=(w == 0), stop=(w == 9))

        while fillers:
            fillers.pop(0)()

        if dbg is not None:
            nc.sync.dma_start(dbg["dbg_qt"][:], qt[:])
            nc.sync.dma_start(dbg["dbg_kt"][:], kt[:])
            nc.sync.dma_start(dbg["dbg_va"][:], va[:])
            nc.sync.dma_start(dbg["dbg_ct"][:], ct[:])


def _in_maps(x, wq_f, wk_f, wv_f, wo_f):
    maps = []
    for core in range(8):
        b, g = core // 4, core % 4
        cols = slice(g * GC, (g + 1) * GC)
        maps.append({
            "xt": np.ascontiguousarray(x[b].T).astype(NP_BF16),
            "wq": np.ascontiguousarray(wq_f[:, cols]).astype(NP_BF16),
            "wk": np.ascontiguousarray(wk_f[:, cols]).astype(NP_BF16),
            "wv": np.ascontiguousarray(wv_f[:, cols]).astype(NP_BF16),
            "wo": np.ascontiguousarray(wo_f[cols, :]).astype(NP_BF16),
        })
    return maps


def _prep(x, Wq, Wk, Wv, Wo, q_scale, k_scale, v_scale, o_scale):
    x = np.asarray(x, dtype=np.float32)
    wq_f = (np.asarray(Wq).T * np.asarray(q_scale).reshape(1, -1)).astype(np.float32)
    wk_f = (np.asarray(Wk).T * np.asarray(k_scale).reshape(1, -1)).astype(np.float32)
    wv_f = (np.asarray(Wv).T * np.asarray(v_scale).reshape(1, -1)).astype(np.float32)
    wo_f = (np.asarray(Wo).T * np.asarray(o_scale).reshape(1, -1)).astype(np.float32)
    return x, wq_f, wk_f, wv_f, wo_f


def run_traced(x, Wq, Wk, Wv, Wo, q_scale, k_scale, v_scale, o_scale):
    """Like kernel() but with NTFF tracing; returns (out, exec_time_ns, trace_path)."""
    x, wq_f, wk_f, wv_f, wo_f = _prep(x, Wq, Wk, Wv, Wo,
                                      q_scale, k_scale, v_scale, o_scale)
    nc = _build()
    res = run_bass_kernel_spmd(nc, _in_maps(x, wq_f, wk_f, wv_f, wo_f),
                               core_ids=list(range(8)), trace=True)
    out = np.zeros((x.shape[0], S, D), dtype=np.float32)
    for core in range(8):
        out[core // 4] += np.asarray(res.results[core]["out_t"],
                                     dtype=np.float32).T
    trace_path = None
    if res.instructions_and_trace is not None:
        trace_path = res.instructions_and_trace[1]
    return out, res.exec_time_ns, trace_path


def kernel(x, Wq, Wk, Wv, Wo, q_scale, k_scale, v_scale, o_scale):
    B = x.shape[0]
    x, wq_f, wk_f, wv_f, wo_f = _prep(x, Wq, Wk, Wv, Wo,
                                      q_scale, k_scale, v_scale, o_scale)
    nc = _build()
    res = run_bass_kernel_spmd(nc, _in_maps(x, wq_f, wk_f, wv_f, wo_f),
                               core_ids=list(range(8)))
    out = np.zeros((B, S, D), dtype=np.float32)
    for core in range(8):
        out[core // 4] += np.asarray(res.results[core]["out_t"],
                                     dtype=np.float32).T
    return out
